# revision 20
# baseline (speedup 1.0000x reference)
"""Trainium2 Bass kernel for nn_ButterflyFilter.

The reference applies, per length-512 row (flattened b*c*angles):
  zero-pad to 1024 -> 10-stage butterfly "FFT" (stage order decreasing)
  -> elementwise filter (bit-reversed order) -> 10-stage butterfly
  "IFFT" (stage order increasing) -> real part of first 512 entries.

Every step is linear in x, so the whole chain is one complex 1024x1024
operator A determined by (twiddle_fft, twiddle_ifft, fourier_filter_br).
Since x is real with support on [:512] and only Re(y)[:512] is kept, the
effective map is the real 512x512 matrix W = Re(A)[:512, :512]:

    proj_row = W @ x_row

x in HBM is (b, c, s, a) — for fixed (b, c) the tile is (s, a), i.e. rows
(angles) are already laid out column-major, exactly the moving-operand
layout the TensorEngine wants. So the device work is 16 independent
512x512x512 matmuls out_bc = W @ x_bc, data-parallel 2 per core across
8 cores. The small parameter-folding (building W from the twiddles) runs
on host in float64; the 32 MiB of row data never touches the host math.

Performance notes (vs the fp32 version):
  * The rel-err gate is 2e-2; fp16 operands + fp32 PSUM accumulation +
    fp16 output store land ~1e-3, so the whole pipeline runs in fp16.
    That halves input DMA (3 MiB -> 1.5 MiB/core) and output DMA
    (2 MiB -> 1 MiB/core). PE pitch at N=512 is one pass for fp32r and
    fp16 alike, so compute time is unchanged — DMA was the bottleneck.
  * Stream order is bc-separated: [W_k|x0_k] fused pieces on the sync
    HWDGE ring, the whole x1 on the scalar ring. bc0's outputs then
    drain while bc1 is still computing; bc1 runs group-sequential
    (o-outer) so its four output chunks stagger into the store queue
    instead of bunching at the end.
  * Block(no_gpsimd_drain=True) skips the expensive gpsimd dge_drain in
    the exit barrier (this kernel issues no SWDGE DMAs).
"""

import os
import sys
import types
from contextlib import ExitStack

import numpy as np

import concourse.bass as bass
import concourse.mybir as mybir
from concourse.bass_utils import run_bass_kernel_spmd


def _ensure_axon_hooks():
    # concourse.bass_utils imports antenv.axon_hooks on the trace path; some
    # images lack that module. Provide a no-op holder so a BASS_TRACE env set
    # by the caller can't crash the run.
    try:
        import antenv.axon_hooks  # noqa: F401
    except Exception:
        m = types.ModuleType("antenv.axon_hooks")
        m._h = None
        m.set_axon_ntff_profile_hook = lambda h: setattr(m, "_h", h)
        m.get_axon_ntff_profile_hook = lambda: m._h
        sys.modules["antenv.axon_hooks"] = m


_ensure_axon_hooks()

N_CORES = 8
S = 512          # input/output row length
NF = 1024        # padded length
P = 128          # SBUF partitions
KC = S // P      # 4 contraction chunks
OC = S // P      # 4 output-row chunks
BC_PER_CORE = 2  # 16 (b,c) tiles / 8 cores

# Exposed for the test harness: exec time of the last device run (ns), if
# profiling was enabled via BUTTERFLY_TRACE=1.
last_exec_time_ns = None
last_results = None


def _butterfly_np(tw, x, increasing):
    # Mirrors the reference butterfly exactly, in numpy (any dtype).
    B, n = x.shape
    m = tw.shape[0]
    order = range(m) if increasing else range(m - 1, -1, -1)
    for idx in order:
        s = 1 << idx
        t = tw[idx].reshape(n // (2 * s), s, 2, 2)
        xr = x.reshape(B, n // (2 * s), 2, s)
        x = np.einsum('gjik,bgkj->bgij', t, xr).reshape(B, n)
    return x


def _compose_wt(twiddle_fft, twiddle_ifft, fourier_filter_br):
    """Fold twiddles+filter into the lhsT operand Wt[i_in, o_out] (512x512 f64)."""
    tw_fft = np.asarray(twiddle_fft, dtype=np.float64)
    tw_ifft = np.asarray(twiddle_ifft, dtype=np.float64)
    filt = np.asarray(fourier_filter_br, dtype=np.float64)
    tf = tw_fft[0, ..., 0] + 1j * tw_fft[0, ..., 1]
    ti = tw_ifft[0, ..., 0] + 1j * tw_ifft[0, ..., 1]
    X = np.eye(NF, dtype=np.complex128)      # row j = e_j
    X = _butterfly_np(tf, X, increasing=False)
    X = X * filt[None, :]
    X = _butterfly_np(ti, X, increasing=True)
    # X = chain(I) = A^T, so X[i, o] = A[o, i]; W[o, i] = Re(A[o, i]).
    # lhsT for out = lhsT.T @ rhs must be Wt[i, o] = W[o, i] = Re(X[i, o]).
    return np.ascontiguousarray(np.real(X[:S, :S]))


def _mm_dtype():
    name = os.environ.get("BUTTERFLY_MM_DTYPE", "fp16")
    return {
        "fp16": (mybir.dt.float16, np.float16),
        "bf16": (mybir.dt.bfloat16, None),  # needs ml_dtypes; fp16 preferred
    }[name]


def _build_nc():
    # Raw Bass (no TileContext): this walrus encodes at most ONE semaphore
    # wait per instruction, which Tile's scheduler and epilogue drain cannot
    # guarantee. With manual engine programs every wait is its own wait_ge.
    #
    # Layout (per core), all fp16:
    #   p[k] (128, 1024) = [W_k | x0_k]: contraction chunk k of the operator
    #   fused with bc-tile-0's chunk, 256 KiB per DMA piece on the sync
    #   ring, so compute starts on the first piece. x1 (128, 2048) is all
    #   of bc-tile-1, one 512 KiB DMA on the scalar ring issued at t=0.
    #   acc[bc*4+o] accumulates in one PSUM bank per group; DVE copies
    #   PSUM fp32 -> SBUF fp16; ACT stores 128 KiB chunks as each group
    #   completes (bc0's groups finish while bc1 computes).
    mmdt, _ = _mm_dtype()
    f32 = mybir.dt.float32
    n_warm = int(os.environ.get("BUTTERFLY_NWARM", "6"))
    no_gpsimd_drain = os.environ.get("BUTTERFLY_GPSIMD_DRAIN", "0") != "1"

    nc = bass.Bass()
    pd = nc.declare_dram_parameter("p", [KC, P, 2 * S], mmdt, isOutput=False)
    x1d = nc.declare_dram_parameter("x1", [KC, P, S], mmdt, isOutput=False)
    # Output stays partition-major ([bc, m, o*512+a]) so multi-group stores
    # are layout-exact copies of the o_sb tiles; host undoes the interleave.
    out = nc.declare_dram_parameter("out", [BC_PER_CORE, P, OC * S], mmdt, isOutput=True)

    with ExitStack() as ctx:
        p_sb = [
            ctx.enter_context(nc.sbuf_tensor(f"p_sb{k}", [P, 2 * S], mmdt))
            for k in range(KC)
        ]
        x1_sb = ctx.enter_context(nc.sbuf_tensor("x1_sb", [P, KC * S], mmdt))
        warm_sb = ctx.enter_context(nc.sbuf_tensor("warm_sb", [P, P + S], mmdt))
        o_sb = [
            ctx.enter_context(nc.sbuf_tensor(f"o_sb{j}", [P, OC * S], mmdt))
            for j in range(BC_PER_CORE)
        ]
        accs = [
            ctx.enter_context(nc.psum_tensor(f"acc{g}", [P, S], f32))
            for g in range(BC_PER_CORE * OC)
        ]
        s_p = [ctx.enter_context(nc.semaphore(f"s_p{k}")) for k in range(KC)]
        s_x1 = [ctx.enter_context(nc.semaphore(f"s_x1{k}")) for k in range(KC)]
        s_pe = ctx.enter_context(nc.semaphore("s_pe"))
        s_cpd = ctx.enter_context(nc.semaphore("s_cpd"))  # DVE copies (even g)
        s_cpa = ctx.enter_context(nc.semaphore("s_cpa"))  # ACT copies (odd g)
        s_out = ctx.enter_context(nc.semaphore("s_out"))
        block = ctx.enter_context(nc.Block(no_gpsimd_drain=no_gpsimd_drain))

        @block.sync
        def _(sync):
            # Input chunks alternate between the SP HWDGE ring (even) and
            # the GPSIMD SWDGE ring (odd): one ring alone streams at only
            # ~160-220 B/ns early on, two together reach ~400 B/ns, so the
            # chunk feed keeps up with the PE. Issue order matches
            # consumption order on each ring. (Putting the odd chunks on
            # the ACT ring instead perturbed ACT's copy/store chain and
            # exposed tail corruption — GPSIMD leaves ACT untouched.)
            for k in range(0, KC, 2):
                sync.dma_start(p_sb[k][:], pd[k]).then_inc(s_p[k], 16)
            for k in range(0, KC, 2):
                sync.dma_start(
                    x1_sb[:, bass.ts(k, S)], x1d[k]
                ).then_inc(s_x1[k], 16)
            # Stores ping-pong between the SP and ACT rings so the ~0.65us
            # per-dma_start issue cost overlaps pairwise. SP: g01, g4, g6.
            sync.wait_ge(s_cpd, 1)
            sync.wait_ge(s_cpa, 1)
            sync.dma_start(out[0, :, : 2 * S], o_sb[0][:, : 2 * S]).then_inc(
                s_out, 16
            )
            sync.wait_ge(s_cpd, 3)
            sync.wait_ge(s_cpa, 3)
            sync.dma_start(out[1, :, : 2 * S], o_sb[1][:, : 2 * S]).then_inc(
                s_out, 16
            )
            # Output stores must have landed in HBM before the NEFF ends;
            # the exit-barrier drain does NOT wait for HWDGE data receipt.
            sync.wait_ge(s_out, 4 * 16)

        @block.scalar
        def _(scalar):
            # ACT interleaves PSUM->SBUF copies of the odd groups (the ACT
            # activation path reads PSUM; GPSIMD cannot) with its share of
            # the stores: bc0b, g5, g7. Every copy->store edge goes through
            # a semaphore — same-engine program order does NOT order the
            # copy's SBUF data against the dma's ring read.
            scalar.wait_ge(s_pe, 2)
            nc.scalar.copy(o_sb[0][:, bass.ts(1, S)], accs[1][:]).then_inc(
                s_cpa, 1
            )
            scalar.wait_ge(s_pe, 4)
            nc.scalar.copy(o_sb[0][:, bass.ts(3, S)], accs[3][:]).then_inc(
                s_cpa, 1
            )
            scalar.wait_ge(s_cpd, 2)
            scalar.wait_ge(s_cpa, 2)
            scalar.dma_start(out[0, :, 2 * S :], o_sb[0][:, 2 * S :]).then_inc(
                s_out, 16
            )
            # Both bc1 copies back-to-back, then one merged 256 KiB store of
            # [g6,g7] — the [g4,g5] half goes out on the SP ring in parallel.
            scalar.wait_ge(s_pe, 6)
            nc.scalar.copy(o_sb[1][:, bass.ts(1, S)], accs[5][:]).then_inc(
                s_cpa, 1
            )
            scalar.wait_ge(s_pe, 8)
            nc.scalar.copy(o_sb[1][:, bass.ts(3, S)], accs[7][:]).then_inc(
                s_cpa, 1
            )
            scalar.wait_ge(s_cpd, 4)
            scalar.wait_ge(s_cpa, 4)
            scalar.dma_start(out[1, :, 2 * S :], o_sb[1][:, 2 * S :]).then_inc(
                s_out, 16
            )

        @block.tensor
        def _(tensor):
            # Warm-up matmuls on an *uninitialized* scratch tile: keeps the
            # PE busy from the first cycle so HAM un-throttles (1.2 -> 2.4
            # GHz) while inputs stream in. Garbage results land in acc 7,
            # which its real accumulation group clears via start=True later;
            # NaNs in the scratch data are harmless to the PE pipeline.
            for _ in range(n_warm):
                nc.tensor.matmul(
                    accs[-1][:], warm_sb[:, :P], warm_sb[:, P:],
                    start=True, stop=True,
                )
            # bc0: k-outer so compute starts on the first 256 KiB piece.
            # The four groups complete during the last piece's o-loop.
            for k in range(KC):
                tensor.wait_ge(s_p[k], 16)
                for o in range(OC):
                    mm = nc.tensor.matmul(
                        accs[o][:],
                        p_sb[k][:, bass.ts(o, P)],
                        p_sb[k][:, S : 2 * S],
                        start=(k == 0),
                        stop=(k == KC - 1),
                    )
                    if k == KC - 1:
                        mm.then_inc(s_pe, 1)
            # bc1: also k-outer, consuming each 128 KiB chunk as it lands.
            for k in range(KC):
                tensor.wait_ge(s_x1[k], 16)
                for o in range(OC):
                    mm = nc.tensor.matmul(
                        accs[OC + o][:],
                        p_sb[k][:, bass.ts(o, P)],
                        x1_sb[:, bass.ts(k, S)],
                        start=(k == 0),
                        stop=(k == KC - 1),
                    )
                    if k == KC - 1:
                        mm.then_inc(s_pe, 1)

        @block.gpsimd
        def _(gpsimd):
            # SWDGE carries the odd input chunks in parallel with the SP
            # ring. GPSIMD does nothing else, so its descriptor generation
            # can't perturb the output drain chain.
            for k in range(1, KC, 2):
                gpsimd.dma_start(p_sb[k][:], pd[k]).then_inc(s_p[k], 16)
            for k in range(1, KC, 2):
                gpsimd.dma_start(
                    x1_sb[:, bass.ts(k, S)], x1d[k]
                ).then_inc(s_x1[k], 16)

        @block.vector
        def _(vector):
            # DVE drains the even groups' PSUM banks.
            for g in range(0, BC_PER_CORE * OC, 2):
                bc, o = divmod(g, OC)
                vector.wait_ge(s_pe, g + 1)
                nc.vector.tensor_copy(
                    o_sb[bc][:, bass.ts(o, S)], accs[g][:]
                ).then_inc(s_cpd, 1)

    return nc


def kernel(x, twiddle_fft, twiddle_ifft, fourier_filter_br):
    global last_exec_time_ns, last_results
    x = np.asarray(x, dtype=np.float32)
    b, c, s_len, a = x.shape
    assert (b, c, s_len, a) == (8, 2, S, S)

    _, npdt = _mm_dtype()
    wt = _compose_wt(twiddle_fft, twiddle_ifft, fourier_filter_br)
    wt4 = wt.reshape(KC, P, S).astype(npdt)
    x16 = x.reshape(b * c, KC, P, S).astype(npdt)  # [bc, k, p, a]

    in_maps = []
    for core in range(N_CORES):
        x0 = x16[BC_PER_CORE * core]
        x1 = x16[BC_PER_CORE * core + 1]
        # p[k] = [w_k | x0_k] along the free dim, one 256 KiB DMA piece each
        pieces = np.concatenate([wt4, x0], axis=2)  # (4, 128, 1024)
        in_maps.append(
            {
                "p": np.ascontiguousarray(pieces),
                "x1": np.ascontiguousarray(x1),  # (4, 128, 512) k-chunks
            }
        )
    nc = _build_nc()
    trace = os.environ.get("BUTTERFLY_TRACE") == "1"
    res = run_bass_kernel_spmd(nc, in_maps, core_ids=list(range(N_CORES)), trace=trace)
    last_exec_time_ns = res.exec_time_ns
    last_results = res

    q = np.concatenate(
        [res.results[k]["out"].astype(np.float32) for k in range(N_CORES)], axis=0
    )
    # q[bc, m, o*512+a]: undo the partition-major store layout to
    # q2[bc, o*128+m, a] = proj.T[o*128+m, bc*512 + a]; reference output is
    # proj.T.reshape(b, c, s, a) — a pure reinterpret of the (512, 8192) buffer.
    q2 = q.reshape(b * c, P, OC, S).transpose(0, 2, 1, 3).reshape(b * c, S, S)
    out = q2.transpose(1, 0, 2).reshape(S, b * c * a).reshape(b, c, s_len, a)
    return np.ascontiguousarray(out)


# revision 22
# speedup vs baseline: 1.1612x; 1.1612x over previous
"""Trainium2 Bass kernel for nn_ButterflyFilter.

The reference applies, per length-512 row (flattened b*c*angles):
  zero-pad to 1024 -> 10-stage butterfly "FFT" (stage order decreasing)
  -> elementwise filter (bit-reversed order) -> 10-stage butterfly
  "IFFT" (stage order increasing) -> real part of first 512 entries.

Every step is linear in x, so the whole chain is one complex 1024x1024
operator A determined by (twiddle_fft, twiddle_ifft, fourier_filter_br).
Since x is real with support on [:512] and only Re(y)[:512] is kept, the
effective map is the real 512x512 matrix W = Re(A)[:512, :512]:

    proj_row = W @ x_row

x in HBM is (b, c, s, a) — for fixed (b, c) the tile is (s, a), i.e. rows
(angles) are already laid out column-major, exactly the moving-operand
layout the TensorEngine wants. So the device work is 16 independent
512x512x512 matmuls out_bc = W @ x_bc, data-parallel 2 per core across
8 cores. The small parameter-folding (building W from the twiddles) runs
on host in float64; the 32 MiB of row data never touches the host math.

Performance notes (vs the fp32 version):
  * The rel-err gate is 2e-2; fp16 operands + fp32 PSUM accumulation +
    fp16 output store land ~1e-3, so the whole pipeline runs in fp16.
    That halves input DMA (3 MiB -> 1.5 MiB/core) and output DMA
    (2 MiB -> 1 MiB/core). PE pitch at N=512 is one pass for fp32r and
    fp16 alike, so compute time is unchanged — DMA was the bottleneck.
  * Stream order is bc-separated: [W_k|x0_k] fused pieces on the sync
    HWDGE ring, the whole x1 on the scalar ring. bc0's outputs then
    drain while bc1 is still computing; bc1 runs group-sequential
    (o-outer) so its four output chunks stagger into the store queue
    instead of bunching at the end.
  * Block(no_gpsimd_drain=True) skips the expensive gpsimd dge_drain in
    the exit barrier (this kernel issues no SWDGE DMAs).
"""

import os
import sys
import types
from contextlib import ExitStack

import numpy as np

import concourse.bass as bass
import concourse.mybir as mybir
from concourse.bass_utils import run_bass_kernel_spmd


def _ensure_axon_hooks():
    # concourse.bass_utils imports antenv.axon_hooks on the trace path; some
    # images lack that module. Provide a no-op holder so a BASS_TRACE env set
    # by the caller can't crash the run.
    try:
        import antenv.axon_hooks  # noqa: F401
    except Exception:
        m = types.ModuleType("antenv.axon_hooks")
        m._h = None
        m.set_axon_ntff_profile_hook = lambda h: setattr(m, "_h", h)
        m.get_axon_ntff_profile_hook = lambda: m._h
        sys.modules["antenv.axon_hooks"] = m


_ensure_axon_hooks()

N_CORES = 8
S = 512          # input/output row length
NF = 1024        # padded length
P = 128          # SBUF partitions
KC = S // P      # 4 contraction chunks
OC = S // P      # 4 output-row chunks
BC_PER_CORE = 2  # 16 (b,c) tiles / 8 cores

# Exposed for the test harness: exec time of the last device run (ns), if
# profiling was enabled via BUTTERFLY_TRACE=1.
last_exec_time_ns = None
last_results = None


def _butterfly_np(tw, x, increasing):
    # Mirrors the reference butterfly exactly, in numpy (any dtype).
    B, n = x.shape
    m = tw.shape[0]
    order = range(m) if increasing else range(m - 1, -1, -1)
    for idx in order:
        s = 1 << idx
        t = tw[idx].reshape(n // (2 * s), s, 2, 2)
        xr = x.reshape(B, n // (2 * s), 2, s)
        x = np.einsum('gjik,bgkj->bgij', t, xr).reshape(B, n)
    return x


def _compose_wt(twiddle_fft, twiddle_ifft, fourier_filter_br):
    """Fold twiddles+filter into the lhsT operand Wt[i_in, o_out] (512x512 f64)."""
    tw_fft = np.asarray(twiddle_fft, dtype=np.float64)
    tw_ifft = np.asarray(twiddle_ifft, dtype=np.float64)
    filt = np.asarray(fourier_filter_br, dtype=np.float64)
    tf = tw_fft[0, ..., 0] + 1j * tw_fft[0, ..., 1]
    ti = tw_ifft[0, ..., 0] + 1j * tw_ifft[0, ..., 1]
    X = np.eye(NF, dtype=np.complex128)      # row j = e_j
    X = _butterfly_np(tf, X, increasing=False)
    X = X * filt[None, :]
    X = _butterfly_np(ti, X, increasing=True)
    # X = chain(I) = A^T, so X[i, o] = A[o, i]; W[o, i] = Re(A[o, i]).
    # lhsT for out = lhsT.T @ rhs must be Wt[i, o] = W[o, i] = Re(X[i, o]).
    return np.ascontiguousarray(np.real(X[:S, :S]))


def _mm_dtype():
    name = os.environ.get("BUTTERFLY_MM_DTYPE", "fp16")
    return {
        "fp16": (mybir.dt.float16, np.float16),
        "bf16": (mybir.dt.bfloat16, None),  # needs ml_dtypes; fp16 preferred
    }[name]


def _build_nc():
    # Raw Bass (no TileContext): this walrus encodes at most ONE semaphore
    # wait per instruction, which Tile's scheduler and epilogue drain cannot
    # guarantee. With manual engine programs every wait is its own wait_ge.
    #
    # Layout (per core), all fp16:
    #   p[k] (128, 1024) = [W_k | x0_k]: contraction chunk k of the operator
    #   fused with bc-tile-0's chunk, 256 KiB per DMA piece on the sync
    #   ring, so compute starts on the first piece. x1 (128, 2048) is all
    #   of bc-tile-1, one 512 KiB DMA on the scalar ring issued at t=0.
    #   acc[bc*4+o] accumulates in one PSUM bank per group; DVE copies
    #   PSUM fp32 -> SBUF fp16; ACT stores 128 KiB chunks as each group
    #   completes (bc0's groups finish while bc1 computes).
    mmdt, _ = _mm_dtype()
    f32 = mybir.dt.float32
    n_warm = int(os.environ.get("BUTTERFLY_NWARM", "6"))
    no_gpsimd_drain = os.environ.get("BUTTERFLY_GPSIMD_DRAIN", "0") != "1"

    nc = bass.Bass()
    pd = nc.declare_dram_parameter("p", [KC, P, 2 * S], mmdt, isOutput=False)
    x1d = nc.declare_dram_parameter("x1", [KC, P, S], mmdt, isOutput=False)
    # Output stays partition-major ([bc, m, o*512+a]) so multi-group stores
    # are layout-exact copies of the o_sb tiles; host undoes the interleave.
    out = nc.declare_dram_parameter("out", [BC_PER_CORE, P, OC * S], mmdt, isOutput=True)

    with ExitStack() as ctx:
        p_sb = [
            ctx.enter_context(nc.sbuf_tensor(f"p_sb{k}", [P, 2 * S], mmdt))
            for k in range(KC)
        ]
        x1_sb = ctx.enter_context(nc.sbuf_tensor("x1_sb", [P, KC * S], mmdt))
        warm_sb = ctx.enter_context(nc.sbuf_tensor("warm_sb", [P, P + S], mmdt))
        o_sb = [
            ctx.enter_context(nc.sbuf_tensor(f"o_sb{j}", [P, OC * S], mmdt))
            for j in range(BC_PER_CORE)
        ]
        accs = [
            ctx.enter_context(nc.psum_tensor(f"acc{g}", [P, S], f32))
            for g in range(BC_PER_CORE * OC)
        ]
        s_p = [ctx.enter_context(nc.semaphore(f"s_p{k}")) for k in range(KC)]
        s_x1 = [ctx.enter_context(nc.semaphore(f"s_x1{k}")) for k in range(KC)]
        s_pe = ctx.enter_context(nc.semaphore("s_pe"))
        s_cpd = ctx.enter_context(nc.semaphore("s_cpd"))  # DVE copies (even g)
        s_cpa = ctx.enter_context(nc.semaphore("s_cpa"))  # ACT copies (odd g)
        s_out = ctx.enter_context(nc.semaphore("s_out"))
        block = ctx.enter_context(nc.Block(no_gpsimd_drain=no_gpsimd_drain))

        @block.sync
        def _(sync):
            # One input ring (SP), issue order = consumption order: the four
            # 256 KiB [W_k|x0_k] pieces, then bc1's four 128 KiB chunks.
            # Splitting inputs across two DMA paths was tried twice and
            # regressed both times: via the ACT ring it perturbed ACT's
            # copy/store chain (tail corruption); via GPSIMD SWDGE the
            # descriptor setup delayed piece 0 and tripped a HAM MID
            # re-throttle. One ring feeds chunks at ~200-300 B/ns, enough
            # to keep the PE within one chunk of saturation.
            for k in range(KC):
                sync.dma_start(p_sb[k][:], pd[k]).then_inc(s_p[k], 16)
            for k in range(KC):
                sync.dma_start(
                    x1_sb[:, bass.ts(k, S)], x1d[k]
                ).then_inc(s_x1[k], 16)
            # Stores ping-pong between the SP and ACT rings so the ~0.65us
            # per-dma_start issue cost overlaps pairwise. SP: g01, g4, g6.
            sync.wait_ge(s_cpd, 1)
            sync.wait_ge(s_cpa, 1)
            sync.dma_start(out[0, :, : 2 * S], o_sb[0][:, : 2 * S]).then_inc(
                s_out, 16
            )
            sync.wait_ge(s_cpd, 3)
            sync.wait_ge(s_cpa, 3)
            sync.dma_start(out[1, :, : 2 * S], o_sb[1][:, : 2 * S]).then_inc(
                s_out, 16
            )
            # Output stores must have landed in HBM before the NEFF ends;
            # the exit-barrier drain does NOT wait for HWDGE data receipt.
            sync.wait_ge(s_out, 4 * 16)

        @block.scalar
        def _(scalar):
            # ACT interleaves PSUM->SBUF copies of the odd groups (the ACT
            # activation path reads PSUM; GPSIMD cannot) with its share of
            # the stores: bc0b, g5, g7. Every copy->store edge goes through
            # a semaphore — same-engine program order does NOT order the
            # copy's SBUF data against the dma's ring read.
            scalar.wait_ge(s_pe, 2)
            nc.scalar.copy(o_sb[0][:, bass.ts(1, S)], accs[1][:]).then_inc(
                s_cpa, 1
            )
            scalar.wait_ge(s_pe, 4)
            nc.scalar.copy(o_sb[0][:, bass.ts(3, S)], accs[3][:]).then_inc(
                s_cpa, 1
            )
            scalar.wait_ge(s_cpd, 2)
            scalar.wait_ge(s_cpa, 2)
            scalar.dma_start(out[0, :, 2 * S :], o_sb[0][:, 2 * S :]).then_inc(
                s_out, 16
            )
            # Both bc1 copies back-to-back, then one merged 256 KiB store of
            # [g6,g7] — the [g4,g5] half goes out on the SP ring in parallel.
            scalar.wait_ge(s_pe, 6)
            nc.scalar.copy(o_sb[1][:, bass.ts(1, S)], accs[5][:]).then_inc(
                s_cpa, 1
            )
            scalar.wait_ge(s_pe, 8)
            nc.scalar.copy(o_sb[1][:, bass.ts(3, S)], accs[7][:]).then_inc(
                s_cpa, 1
            )
            scalar.wait_ge(s_cpd, 4)
            scalar.wait_ge(s_cpa, 4)
            scalar.dma_start(out[1, :, 2 * S :], o_sb[1][:, 2 * S :]).then_inc(
                s_out, 16
            )

        @block.tensor
        def _(tensor):
            # Warm-up matmuls on an *uninitialized* scratch tile: keeps the
            # PE busy from the first cycle so HAM un-throttles (1.2 -> 2.4
            # GHz) while inputs stream in. Garbage results land in acc 7,
            # which its real accumulation group clears via start=True later;
            # NaNs in the scratch data are harmless to the PE pipeline.
            for _ in range(n_warm):
                nc.tensor.matmul(
                    accs[-1][:], warm_sb[:, :P], warm_sb[:, P:],
                    start=True, stop=True,
                )
            # bc0: k-outer so compute starts on the first 256 KiB piece.
            # The four groups complete during the last piece's o-loop.
            for k in range(KC):
                tensor.wait_ge(s_p[k], 16)
                for o in range(OC):
                    mm = nc.tensor.matmul(
                        accs[o][:],
                        p_sb[k][:, bass.ts(o, P)],
                        p_sb[k][:, S : 2 * S],
                        start=(k == 0),
                        stop=(k == KC - 1),
                    )
                    if k == KC - 1:
                        mm.then_inc(s_pe, 1)
            # bc1: also k-outer, consuming each 128 KiB chunk as it lands.
            for k in range(KC):
                tensor.wait_ge(s_x1[k], 16)
                for o in range(OC):
                    mm = nc.tensor.matmul(
                        accs[OC + o][:],
                        p_sb[k][:, bass.ts(o, P)],
                        x1_sb[:, bass.ts(k, S)],
                        start=(k == 0),
                        stop=(k == KC - 1),
                    )
                    if k == KC - 1:
                        mm.then_inc(s_pe, 1)

        @block.vector
        def _(vector):
            # DVE drains the even groups' PSUM banks.
            for g in range(0, BC_PER_CORE * OC, 2):
                bc, o = divmod(g, OC)
                vector.wait_ge(s_pe, g + 1)
                nc.vector.tensor_copy(
                    o_sb[bc][:, bass.ts(o, S)], accs[g][:]
                ).then_inc(s_cpd, 1)

    return nc


def kernel(x, twiddle_fft, twiddle_ifft, fourier_filter_br):
    global last_exec_time_ns, last_results
    x = np.asarray(x, dtype=np.float32)
    b, c, s_len, a = x.shape
    assert (b, c, s_len, a) == (8, 2, S, S)

    _, npdt = _mm_dtype()
    wt = _compose_wt(twiddle_fft, twiddle_ifft, fourier_filter_br)
    wt4 = wt.reshape(KC, P, S).astype(npdt)
    x16 = x.reshape(b * c, KC, P, S).astype(npdt)  # [bc, k, p, a]

    in_maps = []
    for core in range(N_CORES):
        x0 = x16[BC_PER_CORE * core]
        x1 = x16[BC_PER_CORE * core + 1]
        # p[k] = [w_k | x0_k] along the free dim, one 256 KiB DMA piece each
        pieces = np.concatenate([wt4, x0], axis=2)  # (4, 128, 1024)
        in_maps.append(
            {
                "p": np.ascontiguousarray(pieces),
                "x1": np.ascontiguousarray(x1),  # (4, 128, 512) k-chunks
            }
        )
    nc = _build_nc()
    trace = os.environ.get("BUTTERFLY_TRACE") == "1"
    res = run_bass_kernel_spmd(nc, in_maps, core_ids=list(range(N_CORES)), trace=trace)
    last_exec_time_ns = res.exec_time_ns
    last_results = res

    q = np.concatenate(
        [res.results[k]["out"].astype(np.float32) for k in range(N_CORES)], axis=0
    )
    # q[bc, m, o*512+a]: undo the partition-major store layout to
    # q2[bc, o*128+m, a] = proj.T[o*128+m, bc*512 + a]; reference output is
    # proj.T.reshape(b, c, s, a) — a pure reinterpret of the (512, 8192) buffer.
    q2 = q.reshape(b * c, P, OC, S).transpose(0, 2, 1, 3).reshape(b * c, S, S)
    out = q2.transpose(1, 0, 2).reshape(S, b * c * a).reshape(b, c, s_len, a)
    return np.ascontiguousarray(out)
